# revision 1
# baseline (speedup 1.0000x reference)
"""Trainium2 Bass kernel for an 8-expert top-2 MoE layer (B=4, S=2048, D=1024, H=4096).

Strategy (expert-parallel, per the sharding hint):
  - Host computes the router (logits -> top-2 dispatch). This IS the sharding
    function: tokens are gathered per expert ("all-to-all dispatch") and each
    of the 8 NeuronCores receives exactly one expert's token batch + weights.
  - Each core runs the expert FFN on device: h = gelu(x @ w1.T); y = (h @ w2.T) * wt
    with bf16 matmuls accumulating in fp32 PSUM.
  - Host scatter-adds the per-expert outputs back into the full [B,S,D] output
    (the reverse all-to-all / unshard step).

Layouts on device (per core, capacity C tokens padded with zeros):
  xT  [8, 128, C]    bf16   x gathered+transposed, D on partitions (8 chunks of 128)
  w1t [8, 128, 4096] bf16   w1[e].T  (D-major)
  w2t [32, 128, 1024] bf16  w2[e].T  (H-major)
  wt  [128, NT]      f32    routing weight; [p, t] = weight of token t*128+p
  y   [C, 1024]      f32    output (wt already applied; padding rows are 0)
"""

import sys

for _p in ("/opt/trn_rl_repo", "/root/.axon_site/_ro/trn_rl_repo"):
    if _p not in sys.path:
        sys.path.append(_p)

import numpy as np
import ml_dtypes

from concourse import bacc, tile
import concourse.mybir as mybir
from concourse.bass_utils import run_bass_kernel_spmd

BF16 = ml_dtypes.bfloat16
T, D, E, H, TOPK = 8192, 1024, 8, 4096, 2
DCH, HCH = D // 128, H // 128  # 8, 32 partition-chunks

_program_cache: dict = {}


def _build(C: int, repeat: int = 1):
    """Build the SPMD per-core program for capacity C (multiple of 128)."""
    NT = C // 128
    fp32 = mybir.dt.float32
    bf16 = mybir.dt.bfloat16

    nc = bacc.Bacc("TRN2", target_bir_lowering=False, debug=False,
                   enable_asserts=False, num_devices=8)
    xT_d = nc.dram_tensor("xT", [DCH, 128, C], bf16, kind="ExternalInput")
    w1_d = nc.dram_tensor("w1t", [DCH, 128, H], bf16, kind="ExternalInput")
    w2_d = nc.dram_tensor("w2t", [HCH, 128, D], bf16, kind="ExternalInput")
    wt_d = nc.dram_tensor("wt", [128, NT], fp32, kind="ExternalInput")
    y_d = nc.dram_tensor("y", [C, D], fp32, kind="ExternalOutput")

    # token blocks of up to 512 (last may be 128/256/384)
    blocks = []
    off = 0
    while off < C:
        w = min(512, C - off)
        blocks.append((off, w))
        off += w

    with tile.TileContext(nc) as tc:
        with (
            tc.tile_pool(name="wpool", bufs=1) as wpool,
            tc.tile_pool(name="xpool", bufs=2) as xpool,
            tc.tile_pool(name="hpool", bufs=1) as hpool,
            tc.tile_pool(name="ypool", bufs=2) as ypool,
            tc.tile_pool(name="php", bufs=3, space="PSUM") as php,
            tc.tile_pool(name="pyp", bufs=4, space="PSUM") as pyp,
        ):
            w1sb = wpool.tile([128, DCH, H], bf16)
            w2sb = wpool.tile([128, HCH, D], bf16)
            wtsb = wpool.tile([128, NT], fp32)
            for k in range(DCH):
                nc.sync.dma_start(w1sb[:, k, :], w1_d[k])
            for kk in range(HCH):
                nc.sync.dma_start(w2sb[:, kk, :], w2_d[kk])
            nc.sync.dma_start(wtsb[:], wt_d[:])

            for _rep in range(repeat):
                for off, W in blocks:
                    xsb = xpool.tile([128, DCH, W], bf16, tag="x")
                    for k in range(DCH):
                        nc.sync.dma_start(xsb[:, k, :], xT_d[k, :, off:off + W])
                    h = hpool.tile([128, HCH, W], bf16, tag="h")
                    # FC1: h^T[H, tokens] = w1t.T-chunks @ xT  (+ exact GELU)
                    for kk in range(HCH):
                        ph = php.tile([128, W], fp32, tag="ph")
                        for k in range(DCH):
                            nc.tensor.matmul(
                                ph[:],
                                w1sb[:, k, kk * 128:(kk + 1) * 128],
                                xsb[:, k, :],
                                start=(k == 0), stop=(k == DCH - 1),
                            )
                        nc.scalar.activation(
                            h[:, kk, :], ph[:],
                            mybir.ActivationFunctionType.Gelu,
                        )
                    # FC2: y[tokens, D] = h^T-tiles.T @ w2t, scaled by wt
                    for t in range(W // 128):
                        tok = off // 128 + t
                        ysb = ypool.tile([128, D], fp32, tag="y")
                        py0 = pyp.tile([128, 512], fp32, tag="py")
                        py1 = pyp.tile([128, 512], fp32, tag="py")
                        for kk in range(HCH):
                            lhs = h[:, kk, t * 128:(t + 1) * 128]
                            nc.tensor.matmul(py0[:], lhs, w2sb[:, kk, 0:512],
                                             start=(kk == 0), stop=(kk == HCH - 1))
                            nc.tensor.matmul(py1[:], lhs, w2sb[:, kk, 512:1024],
                                             start=(kk == 0), stop=(kk == HCH - 1))
                        sc = wtsb[:, tok:tok + 1]
                        nc.vector.tensor_scalar_mul(ysb[:, 0:512], py0[:], sc)
                        nc.vector.tensor_scalar_mul(ysb[:, 512:1024], py1[:], sc)
                        nc.sync.dma_start(y_d[tok * 128:(tok + 1) * 128, :], ysb[:])

    nc.compile()
    return nc


def _route(x_flat: np.ndarray, router_w: np.ndarray):
    """Host router in fp64 (the sharding/dispatch function).

    Returns (sel[list of per-expert token index arrays], wt[list of f32 weights]).
    Matches the reference's fp32 jax routing: verified gap between 2nd/3rd
    choices on this data far exceeds fp32 rounding noise.
    """
    logits = x_flat.astype(np.float64) @ router_w.T.astype(np.float64)
    # stable argsort of -logits matches jax.lax.top_k tie-breaking (lower index)
    order = np.argsort(-logits, axis=1, kind="stable")
    top2 = order[:, :TOPK]
    m = logits.max(axis=1, keepdims=True)
    p = np.exp(logits - m)
    p /= p.sum(axis=1, keepdims=True)
    sel, wts = [], []
    for e in range(E):
        s = np.nonzero((top2 == e).any(axis=1))[0]
        sel.append(s)
        wts.append(p[s, e].astype(np.float32))
    return sel, wts


def _prepare_in_maps(x_flat, router_w, w1, w2, sel, wts, C):
    NT = C // 128
    in_maps = []
    for e in range(E):
        s = sel[e]
        n = len(s)
        xTb = np.zeros((D, C), dtype=BF16)
        xTb[:, :n] = x_flat[s].T.astype(BF16)
        wtp = np.zeros(C, dtype=np.float32)
        wtp[:n] = wts[e]
        in_maps.append({
            "xT": np.ascontiguousarray(xTb.reshape(DCH, 128, C)),
            "w1t": np.ascontiguousarray(w1[e].T.astype(BF16)).reshape(DCH, 128, H),
            "w2t": np.ascontiguousarray(w2[e].T.astype(BF16)).reshape(HCH, 128, D),
            "wt": np.ascontiguousarray(wtp.reshape(NT, 128).T),
        })
    return in_maps


def kernel(x: np.ndarray, router_w: np.ndarray, w1: np.ndarray, w2: np.ndarray) -> np.ndarray:
    B, S, _ = x.shape
    x_flat = np.asarray(x, np.float32).reshape(T, D)
    router_w = np.asarray(router_w, np.float32)
    w1 = np.asarray(w1, np.float32)
    w2 = np.asarray(w2, np.float32)

    sel, wts = _route(x_flat, router_w)
    max_n = max(len(s) for s in sel)
    C = max(((max_n + 127) // 128) * 128, 128)

    if C not in _program_cache:
        _program_cache[C] = _build(C)
    nc = _program_cache[C]

    in_maps = _prepare_in_maps(x_flat, router_w, w1, w2, sel, wts, C)
    res = run_bass_kernel_spmd(nc, in_maps, list(range(E)))

    out = np.zeros((T, D), np.float32)
    for e in range(E):
        n = len(sel[e])
        out[sel[e]] += res.results[e]["y"][:n]
    return out.reshape(B, S, D)


# revision 5
# speedup vs baseline: 1.1936x; 1.1936x over previous
"""Trainium2 Bass kernel for an 8-expert top-2 MoE layer (B=4, S=2048, D=1024, H=4096).

Strategy (expert-parallel, per the sharding hint):
  - Host computes the router (logits -> top-2 dispatch). This IS the sharding
    function: tokens are gathered per expert ("all-to-all dispatch") and each
    of the 8 NeuronCores receives exactly one expert's token batch + weights.
  - Each core runs the expert FFN on device: h = gelu(x @ w1.T); y = (h @ w2.T) * wt
    with bf16 matmuls accumulating in fp32 PSUM.
  - Host scatter-adds the per-expert outputs back into the full [B,S,D] output
    (the reverse all-to-all / unshard step).

Layouts on device (per core, capacity C tokens padded with zeros):
  xT  [8, 128, C]    bf16   x gathered+transposed, D on partitions (8 chunks of 128)
  w1t [8, 128, 4096] bf16   w1[e].T  (D-major)
  w2t [32, 128, 1024] bf16  w2[e].T  (H-major)
  wt  [128, NT]      f32    routing weight; [p, t] = weight of token t*128+p
  y   [C, 1024]      f32    output (wt already applied; padding rows are 0)
"""

import sys

for _p in ("/opt/trn_rl_repo", "/root/.axon_site/_ro/trn_rl_repo"):
    if _p not in sys.path:
        sys.path.append(_p)

import numpy as np
import ml_dtypes

from concourse import bacc, tile
import concourse.mybir as mybir
from concourse.bass_utils import run_bass_kernel_spmd

BF16 = ml_dtypes.bfloat16
T, D, E, H, TOPK = 8192, 1024, 8, 4096, 2
DCH, HCH = D // 128, H // 128  # 8, 32 partition-chunks

_program_cache: dict = {}


def _build(C: int, repeat: int = 1):
    """Build the SPMD per-core program for capacity C (multiple of 128)."""
    NT = C // 128
    fp32 = mybir.dt.float32
    bf16 = mybir.dt.bfloat16

    nc = bacc.Bacc("TRN2", target_bir_lowering=False, debug=False,
                   enable_asserts=False, num_devices=8)
    xT_d = nc.dram_tensor("xT", [DCH, 128, C], bf16, kind="ExternalInput")
    w1_d = nc.dram_tensor("w1t", [HCH, 128, DCH * 128], bf16, kind="ExternalInput")
    w2_d = nc.dram_tensor("w2t", [HCH, 128, D], bf16, kind="ExternalInput")
    wt_d = nc.dram_tensor("wt", [128, NT], fp32, kind="ExternalInput")
    y_d = nc.dram_tensor("y", [C, D], fp32, kind="ExternalOutput")

    # token blocks of up to 1024, split into halves so each FC1 stationary
    # (w1 tile) serves >=2 matmuls — changing the PE stationary every matmul
    # costs ~573ns/MM vs ~175ns/MM with pair reuse (measured).
    blocks = []  # (offset, [sub-widths])
    off = 0
    while C - off >= 1024:
        blocks.append((off, [512, 512]))
        off += 1024
    rem = C - off
    if rem >= 512:
        blocks.append((off, [rem // 2, rem - rem // 2]))
    elif rem > 0:
        blocks.append((off, [rem]))

    with tile.TileContext(nc) as tc:
        with (
            tc.tile_pool(name="wpool", bufs=1) as wpool,
            tc.tile_pool(name="w1pool", bufs=4) as w1pool,
            tc.tile_pool(name="xpool", bufs=2) as xpool,
            tc.tile_pool(name="hpool", bufs=1) as hpool,
            tc.tile_pool(name="ypool", bufs=2) as ypool,
            tc.tile_pool(name="php", bufs=4, space="PSUM") as php,
            tc.tile_pool(name="pyp", bufs=4, space="PSUM") as pyp,
        ):
            w2sb = wpool.tile([128, HCH, D], bf16)
            wtsb = wpool.tile([128, NT], fp32)
            for kk in range(HCH):
                nc.sync.dma_start(w2sb[:, kk, :], w2_d[kk])
            nc.sync.dma_start(wtsb[:], wt_d[:])

            for _rep in range(repeat):
                for off, subws in blocks:
                    W = sum(subws)
                    xsb = xpool.tile([128, DCH, W], bf16, tag="x")
                    for k in range(DCH):
                        nc.sync.dma_start(xsb[:, k, :], xT_d[k, :, off:off + W])
                    h = hpool.tile([128, HCH, W], bf16, tag="h")
                    # FC1: h^T[H, tokens]; w1 streamed per kk; each stationary
                    # serves one matmul per sub-half (pair reuse)
                    for kk in range(HCH):
                        w1c = w1pool.tile([128, DCH * 128], bf16, tag="w1")
                        nc.sync.dma_start(w1c[:], w1_d[kk])
                        phs = [php.tile([128, w], fp32, tag="ph", name=f"ph{i}")
                               for i, w in enumerate(subws)]
                        for k in range(DCH):
                            lhs = w1c[:, k * 128:(k + 1) * 128]
                            so = 0
                            for ph, w in zip(phs, subws):
                                nc.tensor.matmul(
                                    ph[:], lhs, xsb[:, k, so:so + w],
                                    start=(k == 0), stop=(k == DCH - 1),
                                )
                                so += w
                        so = 0
                        for ph, w in zip(phs, subws):
                            nc.scalar.activation(
                                h[:, kk, so:so + w], ph[:],
                                mybir.ActivationFunctionType.Gelu,
                            )
                            so += w
                    # FC2: y[tokens, D] = h^T-tiles.T @ w2t, scaled by wt
                    for t in range(W // 128):
                        tok = off // 128 + t
                        ysb = ypool.tile([128, D], fp32, tag="y")
                        py0 = pyp.tile([128, 512], fp32, tag="py")
                        py1 = pyp.tile([128, 512], fp32, tag="py")
                        for kk in range(HCH):
                            lhs = h[:, kk, t * 128:(t + 1) * 128]
                            nc.tensor.matmul(py0[:], lhs, w2sb[:, kk, 0:512],
                                             start=(kk == 0), stop=(kk == HCH - 1))
                            nc.tensor.matmul(py1[:], lhs, w2sb[:, kk, 512:1024],
                                             start=(kk == 0), stop=(kk == HCH - 1))
                        sc = wtsb[:, tok:tok + 1]
                        nc.vector.tensor_scalar_mul(ysb[:, 0:512], py0[:], sc)
                        nc.vector.tensor_scalar_mul(ysb[:, 512:1024], py1[:], sc)
                        nc.sync.dma_start(y_d[tok * 128:(tok + 1) * 128, :], ysb[:])

    nc.compile()
    return nc


def _route(x_flat: np.ndarray, router_w: np.ndarray):
    """Host router in fp64 (the sharding/dispatch function).

    Returns (sel[list of per-expert token index arrays], wt[list of f32 weights]).
    Matches the reference's fp32 jax routing: verified gap between 2nd/3rd
    choices on this data far exceeds fp32 rounding noise.
    """
    logits = x_flat.astype(np.float64) @ router_w.T.astype(np.float64)
    # stable argsort of -logits matches jax.lax.top_k tie-breaking (lower index)
    order = np.argsort(-logits, axis=1, kind="stable")
    top2 = order[:, :TOPK]
    m = logits.max(axis=1, keepdims=True)
    p = np.exp(logits - m)
    p /= p.sum(axis=1, keepdims=True)
    sel, wts = [], []
    for e in range(E):
        s = np.nonzero((top2 == e).any(axis=1))[0]
        sel.append(s)
        wts.append(p[s, e].astype(np.float32))
    return sel, wts


def _prepare_in_maps(x_flat, router_w, w1, w2, sel, wts, C):
    NT = C // 128
    in_maps = []
    for e in range(E):
        s = sel[e]
        n = len(s)
        xTb = np.zeros((D, C), dtype=BF16)
        xTb[:, :n] = x_flat[s].T.astype(BF16)
        wtp = np.zeros(C, dtype=np.float32)
        wtp[:n] = wts[e]
        w1r = (w1[e].T.astype(BF16).reshape(DCH, 128, HCH, 128)
               .transpose(2, 1, 0, 3).reshape(HCH, 128, DCH * 128))
        in_maps.append({
            "xT": np.ascontiguousarray(xTb.reshape(DCH, 128, C)),
            "w1t": np.ascontiguousarray(w1r),
            "w2t": np.ascontiguousarray(w2[e].T.astype(BF16)).reshape(HCH, 128, D),
            "wt": np.ascontiguousarray(wtp.reshape(NT, 128).T),
        })
    return in_maps


def kernel(x: np.ndarray, router_w: np.ndarray, w1: np.ndarray, w2: np.ndarray) -> np.ndarray:
    B, S, _ = x.shape
    x_flat = np.asarray(x, np.float32).reshape(T, D)
    router_w = np.asarray(router_w, np.float32)
    w1 = np.asarray(w1, np.float32)
    w2 = np.asarray(w2, np.float32)

    sel, wts = _route(x_flat, router_w)
    max_n = max(len(s) for s in sel)
    C = max(((max_n + 127) // 128) * 128, 128)

    if C not in _program_cache:
        _program_cache[C] = _build(C)
    nc = _program_cache[C]

    in_maps = _prepare_in_maps(x_flat, router_w, w1, w2, sel, wts, C)
    res = run_bass_kernel_spmd(nc, in_maps, list(range(E)))

    out = np.zeros((T, D), np.float32)
    for e in range(E):
        n = len(sel[e])
        out[sel[e]] += res.results[e]["y"][:n]
    return out.reshape(B, S, D)
